# revision 21
# baseline (speedup 1.0000x reference)
"""Multi-head attention (B=4, N=2048, D=1024, H=16) on 8 TRN2 NeuronCores.

Sharding: 8 cores = batch(4) x sequence-half(2). Each core computes the full
attention output for its 1024-token slice of one batch (all 16 heads), so the
final unshard is a pure gather. The only cross-core traffic is an AllGather of
K^T and V between the two cores of each batch pair.

Per-core pipeline (bf16 matmul operands, fp32 PSUM accumulation):
  1. Cast x / w_qkv / w_proj to bf16, stage to DRAM column-blocked, and
     DMA-transpose back so contraction dims sit on SBUF partitions.
     Loads + all collective-adjacent DMAs live on the gpsimd queue (whose
     program order matches data-readiness order); transposes on sync;
     staging stores on scalar.
  2. QKV projection. Q^T and K^T are produced in [d_out, token] orientation;
     V in natural [token, d] orientation, written into a per-head padded
     layout [V_h | 1] (65 cols per head) so the attention O-matmul also
     produces the softmax denominator.
  3. AllGather K^T then V across the pair (k-token axis spans both halves).
  4. Attention per head-pair p: S^T = (QK^T)^T via row-paired matmuls
     (contraction = head_dim 64, two heads in array row halves), exp on
     ScalarE straight out of PSUM (logits are bounded, no max subtraction),
     then one matmul per head with lhsT = [V_h | ones-col] producing
     [O_h^T; dn_h] in 65 PSUM rows - no separate denominator matmuls.
     Normalization: reciprocal of the dn row, partition-broadcast via a
     stride-0 DMA, then a single fused multiply writing bf16. The h=1 head's
     output hops through an SBUF temp + DMA to land on partitions 64-127.
  5. Output projection from the accumulated attout^T tiles, bias add, DMA out.
"""

import sys

for _p in ("/opt/trn_rl_repo",):
    if _p not in sys.path:
        sys.path.insert(0, _p)

import numpy as np

import concourse.bass as bass
import concourse.mybir as mybir
import concourse.tile as tile
from concourse import bacc
from concourse.bass_utils import run_bass_kernel_spmd

B, N, D, H, HD = 4, 2048, 1024, 16, 64
SCALE = HD ** -0.5
NL = N // 2  # tokens per core
VP = 65      # padded head width in the V layout: [V_h (64) | ones (1)]
DV = 16 * VP  # 1040
NCORES = 8
RG = [[0, 1], [2, 3], [4, 5], [6, 7]]
F32 = mybir.dt.float32
BF16 = mybir.dt.bfloat16
EXP = mybir.ActivationFunctionType.Exp


def _dram_bcast(ap_1d, n):
    """Read a contiguous DRAM row replicated across n partitions."""
    return bass.AP(tensor=ap_1d.tensor, offset=ap_1d.offset,
                   ap=[[0, n]] + list(ap_1d.ap))


def _emit(tc, aps):
    nc = tc.nc
    x_l, wqkv, wproj, bias, out = (
        aps["x_local"], aps["w_qkv"], aps["w_proj"], aps["b_proj"], aps["out"])
    x_blk, wqkv_blk, wproj_blk = aps["x_blk"], aps["wqkv_blk"], aps["wproj_blk"]
    cc_k, cc_v, k_g, v_g = aps["cc_k"], aps["cc_v"], aps["k_g"], aps["v_g"]
    rc_d = aps["rc_dram"]

    persist1 = tc.alloc_tile_pool(name="persist1", bufs=1)
    # Shared PSUM pool: s_ps (2 x 2 banks) serves the QKV/output projections
    # and the attention S tiles; o_ps (4 x 1 bank) holds [O_h; dn_h].
    attps = tc.alloc_tile_pool(name="att_ps", bufs=1, space="PSUM")

    # ---- Phase A: load fp32, cast bf16, stage to DRAM column-blocked ------
    # (one [rows, 128] contiguous block per k-tile so the DMA-transposes
    # read contiguous DRAM). All input loads issue up-front on gpsimd in
    # consumption order (x, V, K, Q, wproj) so HBM streams continuously;
    # casts on vector; blocked stores on scalar; transposes on sync.
    qkvp = tc.alloc_tile_pool(name="qkvp", bufs=1)
    qkvsb = tc.alloc_tile_pool(name="qkvsb", bufs=2)
    prep = tc.alloc_tile_pool(name="prep", bufs=1)

    def load_tiles(src, tiles):
        lds = []
        for i in tiles:
            t = prep.tile([128, D], F32, tag="ld_f32", bufs=6, name=f"ld{i}")
            nc.gpsimd.dma_start(out=t, in_=src[i * 128:(i + 1) * 128, :])
            lds.append(t)
        return lds

    def cast_store(lds, blk, tiles):
        for t, i in zip(lds, tiles):
            tb = prep.tile([128, D], BF16, tag="cast_bf", bufs=3, name=f"cb{i}")
            nc.vector.tensor_copy(tb, t)
            dst = bass.AP(tensor=blk.tensor,
                          offset=blk.offset + i * 128 * 128,
                          ap=[[128, 128], [blk.ap[0][0], 8], [1, 128]])
            nc.scalar.dma_start(out=dst, in_=tb.rearrange("p (k c) -> p k c", k=8))

    bias_sb = persist1.tile([128, D], F32, tag="bias")
    bias_bcast = bass.AP(tensor=bias.tensor, offset=bias.offset,
                         ap=[[0, 128], *bias.ap])
    nc.scalar.dma_start(out=bias_sb, in_=bias_bcast)

    qT = [persist1.tile([128, NL], BF16, tag=f"qT{p}", name=f"qT{p}") for p in range(8)]
    kT = [persist1.tile([128, N], BF16, tag=f"kT{p}", name=f"kT{p}") for p in range(8)]
    vvA = [persist1.tile([128, DV // 2], BF16, tag=f"vA{kt}", name=f"vA{kt}")
           for kt in range(16)]
    vvB = [persist1.tile([128, DV // 2], BF16, tag=f"vB{kt}", name=f"vB{kt}")
           for kt in range(16)]
    wpT = [persist1.tile([128, D], BF16, tag=f"wpT{k}", name=f"wpT{k}")
           for k in range(8)]

    xT = [qkvp.tile([128, NL], BF16, tag=f"xT{k}", name=f"xT{k}") for k in range(8)]
    wT = [qkvp.tile([128, 3 * D], BF16, tag=f"wT{k}", name=f"wT{k}") for k in range(8)]

    def wT_load(lo):
        for k in range(8):
            nc.sync.dma_start_transpose(
                out=wT[k][:, lo:lo + 1024], in_=wqkv_blk[k, lo:lo + 1024, :])

    # x/V/K loads first (gpsimd) in consumption order; Q/wproj loads are
    # emitted after the V collectives so the cc_v stores are not queued
    # behind the whole load stream on the in-order gpsimd queue
    ld_x = load_tiles(x_l, range(8))
    ld_v = load_tiles(wqkv, range(16, 24))   # V rows 2048:3072
    ld_k = load_tiles(wqkv, range(8, 16))    # K rows 1024:2048

    cast_store(ld_x, x_blk, range(8))
    for k in range(8):
        nc.sync.dma_start_transpose(out=xT[k], in_=x_blk[k])
    cast_store(ld_v, wqkv_blk, range(16, 24))
    wT_load(2048)
    cast_store(ld_k, wqkv_blk, range(8, 16))
    wT_load(1024)

    # ---- V projection first: its AllGather (split in head-halves so each
    # 1MB piece transfers early) is the longest-latency collective.
    # Written into the padded per-head layout [V_h | 1] so the ones column
    # rides through the AllGather.
    for t in range(8):
        vsb = qkvsb.tile([128, DV], BF16, tag="v_loc")
        v4 = vsb.rearrange("p (pp h c) -> p pp h c", pp=8, h=2, c=VP)
        # full-tile memset: the projection copies overwrite everything except
        # the per-head ones column (col 64 of each 65-wide head slot)
        nc.vector.memset(vsb, 1.0)
        ps = attps.tile([128, 2, 512], F32, tag="s_ps", bufs=2)
        for k in range(8):
            for vc in range(2):
                nc.tensor.matmul(
                    out=ps[:, vc, :],
                    lhsT=xT[k][:, t * 128:(t + 1) * 128],
                    rhs=wT[k][:, 2 * D + vc * 512:2 * D + (vc + 1) * 512],
                    start=(k == 0), stop=(k == 7))
        for vc in range(2):
            nc.vector.tensor_copy(
                v4[:, vc * 4:(vc + 1) * 4, :, 0:64],
                ps[:, vc, :].rearrange("p (a h c) -> p a h c", a=4, h=2, c=64))
            cc_half = (cc_v[0] if vc == 0 else cc_v[1])
            nc.gpsimd.dma_start(
                out=cc_half[t * 128:(t + 1) * 128, :],
                in_=vsb[:, vc * (DV // 2):(vc + 1) * (DV // 2)])
    for half in range(2):
        nc.gpsimd.collective_compute(
            "AllGather", mybir.AluOpType.bypass, replica_groups=RG,
            ins=[cc_v[half]], outs=[v_g[half]])

    ld_q = load_tiles(wqkv, range(0, 8))     # Q rows 0:1024
    ld_w = load_tiles(wproj, range(8))
    cast_store(ld_q, wqkv_blk, range(0, 8))
    wT_load(0)
    cast_store(ld_w, wproj_blk, range(8))
    for k in range(8):
        nc.sync.dma_start_transpose(out=wpT[k], in_=wproj_blk[k])
    prep.release()

    # gathered V loads on sync (idle after the transposes)
    for kt in range(16):
        nc.sync.dma_start(
            out=vvA[kt], in_=v_g[0][kt // 8, (kt % 8) * 128:(kt % 8 + 1) * 128, :])
    for kt in range(16):
        nc.sync.dma_start(
            out=vvB[kt], in_=v_g[1][kt // 8, (kt % 8) * 128:(kt % 8 + 1) * 128, :])

    def proj_dT(m, dst_sb):
        ps = attps.tile([128, 2, 512], F32, tag="s_ps", bufs=2)
        for k in range(8):
            for qc in range(2):
                nc.tensor.matmul(
                    out=ps[:, qc, :],
                    lhsT=wT[k][:, m * 128:(m + 1) * 128],
                    rhs=xT[k][:, qc * 512:(qc + 1) * 512],
                    start=(k == 0), stop=(k == 7))
        # single contiguous [128,1024] PSUM->SBUF copy on ScalarE (Copy is in
        # every ACT table set, so no table reload before the exps)
        nc.scalar.copy(dst_sb, ps.rearrange("p a c -> p (a c)"))

    # ---- K projection + AllGather (fast mesh path)
    for m in range(8, 16):
        ksb = qkvsb.tile([128, NL], BF16, tag="k_loc")
        proj_dT(m, ksb)
        nc.gpsimd.dma_start(out=cc_k[(m - 8) * 128:(m - 7) * 128, :], in_=ksb)
    nc.gpsimd.collective_compute(
        "AllGather", mybir.AluOpType.bypass, replica_groups=RG,
        ins=[cc_k], outs=[k_g])
    # gathered K loads on scalar (between the projection copies and the exps)
    for p in range(8):
        nc.scalar.dma_start(out=kT[p][:, 0:NL], in_=k_g[0, p * 128:(p + 1) * 128, :])
        nc.scalar.dma_start(out=kT[p][:, NL:N], in_=k_g[1, p * 128:(p + 1) * 128, :])

    # ---- Q projection (attention starts as qT tiles stream out)
    for m in range(8):
        proj_dT(m, qT[m])

    qkvsb.release()
    qkvp.release()

    # ---- Phase D: attention ----------------------------------------------
    persist2 = tc.alloc_tile_pool(name="persist2", bufs=1)
    attoutT = [persist2.tile([128, NL], BF16, tag=f"ao{p}", name=f"ao{p}") for p in range(8)]

    with tc.tile_pool(name="pT", bufs=4) as ppool, \
         tc.tile_pool(name="rcp", bufs=3) as rcpool, \
         tc.tile_pool(name="rcb", bufs=3) as rbpool, \
         tc.tile_pool(name="tmp", bufs=3) as tmppool:
        for p in range(8):
            for qc in range(2):
                qsl = slice(qc * 512, (qc + 1) * 512)
                oh = [attps.tile([65, 512], F32, tag="o_ps", bufs=4,
                                 name=f"oh{h}") for h in range(2)]
                for kt in range(16):
                    s = attps.tile([128, 2, 512], F32, tag="s_ps", bufs=2)
                    for h in range(2):
                        nc.tensor.matmul(
                            out=s[:, h, :],
                            lhsT=kT[p][h * 64:(h + 1) * 64, kt * 128:(kt + 1) * 128],
                            rhs=qT[p][h * 64:(h + 1) * 64, qsl],
                            start=True, stop=True,
                            tile_position=(h * 64, 0))
                    pt = ppool.tile([128, 2, 512], BF16, tag="pT")
                    nc.scalar.activation(pt, s, EXP, scale=SCALE)
                    vhalf = vvA if p < 4 else vvB
                    pp = p % 4
                    for h in range(2):
                        nc.tensor.matmul(
                            out=oh[h],
                            lhsT=vhalf[kt][:, pp * 130 + h * VP:pp * 130 + (h + 1) * VP],
                            rhs=pt[:, h, :],
                            start=(kt == 0), stop=(kt == 15))
                # Normalization: reciprocal of the dn row (PSUM partition 64),
                # broadcast across 64 partitions via stride-0 DMA, fused mul.
                rc = [rcpool.tile([65, 512], F32, tag=f"rc{h}", name=f"rc{h}")
                      for h in range(2)]
                rb = [rbpool.tile([64, 512], F32, tag=f"rb{h}", name=f"rb{h}")
                      for h in range(2)]
                for h in range(2):
                    nc.vector.reciprocal(rc[h][64:65, :], oh[h][64:65, :])
                    nc.gpsimd.dma_start(out=rc_d[p, qc, h], in_=rc[h][64:65, :])
                    nc.gpsimd.dma_start(out=rb[h], in_=_dram_bcast(rc_d[p, qc, h], 64))
                nc.vector.tensor_mul(attoutT[p][0:64, qsl], oh[0][0:64, :], rb[0])
                tmp = tmppool.tile([64, 512], BF16, tag="tmp")
                nc.vector.tensor_mul(tmp, oh[1][0:64, :], rb[1])
                nc.gpsimd.dma_start(out=attoutT[p][64:128, qsl], in_=tmp)

    # ---- Phase E: output projection + bias --------------------------------
    with tc.tile_pool(name="y_sb", bufs=3) as ypool:
        for tt in range(8):
            ps = attps.tile([128, 2, 512], F32, tag="s_ps", bufs=2)
            for p in range(8):
                for ec in range(2):
                    nc.tensor.matmul(
                        out=ps[:, ec, :],
                        lhsT=attoutT[p][:, tt * 128:(tt + 1) * 128],
                        rhs=wpT[p][:, ec * 512:(ec + 1) * 512],
                        start=(p == 0), stop=(p == 7))
            yt = ypool.tile([128, D], F32, tag="y_sb")
            for ec in range(2):
                nc.vector.tensor_add(yt[:, ec * 512:(ec + 1) * 512], ps[:, ec, :],
                                     bias_sb[:, ec * 512:(ec + 1) * 512])
            nc.sync.dma_start(out=out[tt * 128:(tt + 1) * 128, :], in_=yt)
    persist2.release()
    attps.release()
    persist1.release()


def _build():
    nc = bacc.Bacc("TRN2", target_bir_lowering=False, debug=False,
                   num_devices=NCORES)
    aps = {
        "x_local": nc.dram_tensor("x_local", [NL, D], F32, kind="ExternalInput").ap(),
        "w_qkv": nc.dram_tensor("w_qkv", [3 * D, D], F32, kind="ExternalInput").ap(),
        "w_proj": nc.dram_tensor("w_proj", [D, D], F32, kind="ExternalInput").ap(),
        "b_proj": nc.dram_tensor("b_proj", [D], F32, kind="ExternalInput").ap(),
        "out": nc.dram_tensor("out", [NL, D], F32, kind="ExternalOutput").ap(),
        "wqkv_blk": nc.dram_tensor("wqkv_blk", [8, 3 * D, 128], BF16).ap(),
        "wproj_blk": nc.dram_tensor("wproj_blk", [8, D, 128], BF16).ap(),
        "x_blk": nc.dram_tensor("x_blk", [8, NL, 128], BF16).ap(),
        "cc_k": nc.dram_tensor("cc_k", [D, NL], BF16).ap(),
        "cc_v": [nc.dram_tensor(f"cc_v{i}", [NL, DV // 2], BF16).ap()
                 for i in range(2)],
        "k_g": nc.dram_tensor("k_g", [2, D, NL], BF16).ap(),
        "v_g": [nc.dram_tensor(f"v_g{i}", [2, NL, DV // 2], BF16).ap()
                for i in range(2)],
        "rc_dram": nc.dram_tensor("rc_dram", [8, 2, 2, 512], F32).ap(),
    }
    with tile.TileContext(nc) as tc:
        _emit(tc, aps)
    nc.compile()
    return nc


_NC = None


def _get_nc():
    global _NC
    if _NC is None:
        _NC = _build()
    return _NC


def run(x, w_qkv, w_proj, b_proj, **spmd_kwargs):
    nc = _get_nc()
    x = np.ascontiguousarray(np.asarray(x, dtype=np.float32))
    w_qkv = np.ascontiguousarray(np.asarray(w_qkv, dtype=np.float32))
    w_proj = np.ascontiguousarray(np.asarray(w_proj, dtype=np.float32))
    b_proj = np.ascontiguousarray(np.asarray(b_proj, dtype=np.float32))
    in_maps = []
    for c in range(NCORES):
        b, half = divmod(c, 2)
        in_maps.append({
            "x_local": np.ascontiguousarray(x[b, half * NL:(half + 1) * NL, :]),
            "w_qkv": w_qkv,
            "w_proj": w_proj,
            "b_proj": b_proj,
        })
    res = run_bass_kernel_spmd(nc, in_maps, list(range(NCORES)), **spmd_kwargs)
    y = np.empty((B, N, D), dtype=np.float32)
    for c in range(NCORES):
        b, half = divmod(c, 2)
        y[b, half * NL:(half + 1) * NL, :] = res.results[c]["out"]
    return y, res


def kernel(x, w_qkv, w_proj, b_proj):
    y, _ = run(x, w_qkv, w_proj, b_proj)
    return y


# revision 28
# speedup vs baseline: 1.2453x; 1.2453x over previous
"""Multi-head attention (B=4, N=2048, D=1024, H=16) on 8 TRN2 NeuronCores.

Sharding: 8 cores = batch(4) x sequence-half(2). Each core computes the full
attention output for its 1024-token slice of one batch (all 16 heads), so the
final unshard is a pure gather. The only cross-core traffic is an AllGather of
K^T and V between the two cores of each batch pair.

Per-core pipeline (bf16 matmul operands, fp32 PSUM accumulation):
  1. Cast x / w_qkv / w_proj to bf16, stage to DRAM column-blocked, and
     DMA-transpose back so contraction dims sit on SBUF partitions.
     Loads + all collective-adjacent DMAs live on the gpsimd queue (whose
     program order matches data-readiness order); transposes on sync;
     staging stores on scalar.
  2. QKV projection. Q^T and K^T are produced in [d_out, token] orientation;
     V in natural [token, d] orientation, written into a per-head padded
     layout [V_h | 1] (65 cols per head) so the attention O-matmul also
     produces the softmax denominator.
  3. AllGather K^T then V across the pair (k-token axis spans both halves).
  4. Attention per head-pair p: S^T = (QK^T)^T via row-paired matmuls
     (contraction = head_dim 64, two heads in array row halves), exp on
     ScalarE straight out of PSUM (logits are bounded, no max subtraction),
     then one matmul per head with lhsT = [V_h | ones-col] producing
     [O_h^T; dn_h] in 65 PSUM rows - no separate denominator matmuls.
     Normalization: reciprocal of the dn row, partition-broadcast via a
     stride-0 DMA, then a single fused multiply writing bf16. The h=1 head's
     output hops through an SBUF temp + DMA to land on partitions 64-127.
  5. Output projection from the accumulated attout^T tiles, bias add, DMA out.
"""

import sys

for _p in ("/opt/trn_rl_repo",):
    if _p not in sys.path:
        sys.path.insert(0, _p)

import numpy as np

import concourse.bass as bass
import concourse.mybir as mybir
import concourse.tile as tile
from concourse import bacc
from concourse.bass_utils import run_bass_kernel_spmd

B, N, D, H, HD = 4, 2048, 1024, 16, 64
SCALE = HD ** -0.5
NL = N // 2  # tokens per core
VP = 65      # padded head width in the V layout: [V_h (64) | ones (1)]
DV = 16 * VP  # 1040
NCORES = 8
RG = [[0, 1], [2, 3], [4, 5], [6, 7]]
F32 = mybir.dt.float32
BF16 = mybir.dt.bfloat16
EXP = mybir.ActivationFunctionType.Exp


def _dram_bcast(ap_1d, n):
    """Read a contiguous DRAM row replicated across n partitions."""
    return bass.AP(tensor=ap_1d.tensor, offset=ap_1d.offset,
                   ap=[[0, n]] + list(ap_1d.ap))


def _emit(tc, aps):
    nc = tc.nc
    x_l, wqkv, wproj, bias, out = (
        aps["x_local"], aps["w_qkv"], aps["w_proj"], aps["b_proj"], aps["out"])
    wproj_blk = aps["wproj_blk"]
    cc_k, cc_v, k_g, v_g = aps["cc_k"], aps["cc_v"], aps["k_g"], aps["v_g"]
    rc_d = aps["rc_dram"]

    persist1 = tc.alloc_tile_pool(name="persist1", bufs=1)
    # Shared PSUM pool: s_ps (2 x 2 banks) serves the QKV/output projections
    # and the attention S tiles; o_ps (4 x 1 bank) holds [O_h; dn_h].
    attps = tc.alloc_tile_pool(name="att_ps", bufs=1, space="PSUM")

    # ---- Phase A: load fp32, transpose on the TensorE (transpose-mode
    # matmul against an inline identity - PE is otherwise idle here), cast
    # to bf16 in the PSUM->SBUF copy-out on VectorE. No DRAM staging
    # round-trip and no xbar-transpose serialization for x / w_qkv; only
    # w_proj (needed ~400us later) keeps the staged xbar path.
    qkvp = tc.alloc_tile_pool(name="qkvp", bufs=1)
    qkvsb = tc.alloc_tile_pool(name="qkvsb", bufs=2)
    prep = tc.alloc_tile_pool(name="prep", bufs=1)

    def load_tiles(src, tiles):
        lds = []
        for i in tiles:
            t = prep.tile([128, D], F32, tag="ld_f32", bufs=6, name=f"ld{i}")
            nc.gpsimd.dma_start(out=t, in_=src[i * 128:(i + 1) * 128, :])
            lds.append(t)
        return lds

    bias_sb = persist1.tile([128, D], F32, tag="bias")
    bias_bcast = bass.AP(tensor=bias.tensor, offset=bias.offset,
                         ap=[[0, 128], *bias.ap])
    nc.scalar.dma_start(out=bias_sb, in_=bias_bcast)

    ident = persist1.tile([128, 128], F32, tag="ident")
    nc.scalar.dma_start(out=ident, in_=aps["ident_dram"])

    def pe_transpose(lds, dst, base):
        """Transpose 8 [128, D] f32 tiles into dst[k][:, base + r*128] bf16.

        Four 128x128 transpose-mode matmuls share one PSUM bank, then one
        [128,512] VectorE copy casts to bf16. Row-quad outer so only 4 source
        tiles are live at a time.
        """
        for rq in range(2):
            for k in range(8):
                tps = attps.tile([128, 512], F32, tag="o_ps", bufs=4, name="tps")
                for j in range(4):
                    nc.tensor.transpose(
                        tps[:, j * 128:(j + 1) * 128],
                        lds[rq * 4 + j][:, k * 128:(k + 1) * 128], ident)
                nc.vector.tensor_copy(
                    dst[k][:, base + rq * 512:base + (rq + 1) * 512], tps)

    qT = [persist1.tile([128, NL], BF16, tag=f"qT{p}", name=f"qT{p}") for p in range(8)]
    kT = [persist1.tile([128, N], BF16, tag=f"kT{p}", name=f"kT{p}") for p in range(8)]
    vvA = [persist1.tile([128, DV // 2], BF16, tag=f"vA{kt}", name=f"vA{kt}")
           for kt in range(16)]
    vvB = [persist1.tile([128, DV // 2], BF16, tag=f"vB{kt}", name=f"vB{kt}")
           for kt in range(16)]
    wpT = [persist1.tile([128, D], BF16, tag=f"wpT{k}", name=f"wpT{k}")
           for k in range(8)]

    xT = [qkvp.tile([128, NL], BF16, tag=f"xT{k}", name=f"xT{k}") for k in range(8)]
    wT = [qkvp.tile([128, 3 * D], BF16, tag=f"wT{k}", name=f"wT{k}") for k in range(8)]

    # loads issue up-front on gpsimd in consumption order
    ld_x = load_tiles(x_l, range(8))
    ld_v = load_tiles(wqkv, range(16, 24))   # V rows 2048:3072
    ld_k = load_tiles(wqkv, range(8, 16))    # K rows 1024:2048
    ld_q = load_tiles(wqkv, range(0, 8))     # Q rows 0:1024

    pe_transpose(ld_x, xT, 0)
    pe_transpose(ld_v, wT, 2048)

    # ---- V projection first: its AllGather (split in head-halves so each
    # 1MB piece transfers early) is the longest-latency collective.
    # Written into the padded per-head layout [V_h | 1] so the ones column
    # rides through the AllGather.
    for t in range(8):
        vsb = qkvsb.tile([128, DV], BF16, tag="v_loc")
        v4 = vsb.rearrange("p (pp h c) -> p pp h c", pp=8, h=2, c=VP)
        # full-tile memset: the projection copies overwrite everything except
        # the per-head ones column (col 64 of each 65-wide head slot)
        nc.vector.memset(vsb, 1.0)
        ps = attps.tile([128, 2, 512], F32, tag="s_ps", bufs=2)
        for k in range(8):
            for vc in range(2):
                nc.tensor.matmul(
                    out=ps[:, vc, :],
                    lhsT=xT[k][:, t * 128:(t + 1) * 128],
                    rhs=wT[k][:, 2 * D + vc * 512:2 * D + (vc + 1) * 512],
                    start=(k == 0), stop=(k == 7))
        for vc in range(2):
            nc.vector.tensor_copy(
                v4[:, vc * 4:(vc + 1) * 4, :, 0:64],
                ps[:, vc, :].rearrange("p (a h c) -> p a h c", a=4, h=2, c=64))
            cc_half = (cc_v[0] if vc == 0 else cc_v[1])
            nc.gpsimd.dma_start(
                out=cc_half[t * 128:(t + 1) * 128, :],
                in_=vsb[:, vc * (DV // 2):(vc + 1) * (DV // 2)])
    for half in range(2):
        nc.gpsimd.collective_compute(
            "AllGather", mybir.AluOpType.bypass, replica_groups=RG,
            ins=[cc_v[half]], outs=[v_g[half]])

    # gathered V loads on sync (free of transposes now)
    for kt in range(16):
        nc.sync.dma_start(
            out=vvA[kt], in_=v_g[0][kt // 8, (kt % 8) * 128:(kt % 8 + 1) * 128, :])
    for kt in range(16):
        nc.sync.dma_start(
            out=vvB[kt], in_=v_g[1][kt // 8, (kt % 8) * 128:(kt % 8 + 1) * 128, :])

    pe_transpose(ld_k, wT, 1024)

    def proj_dT(m, dst_sb):
        ps = attps.tile([128, 2, 512], F32, tag="s_ps", bufs=2)
        for k in range(8):
            for qc in range(2):
                nc.tensor.matmul(
                    out=ps[:, qc, :],
                    lhsT=wT[k][:, m * 128:(m + 1) * 128],
                    rhs=xT[k][:, qc * 512:(qc + 1) * 512],
                    start=(k == 0), stop=(k == 7))
        # single contiguous [128,1024] PSUM->SBUF copy on ScalarE (Copy is in
        # every ACT table set, so no table reload before the exps)
        nc.scalar.copy(dst_sb, ps.rearrange("p a c -> p (a c)"))

    # ---- K projection + AllGather (fast mesh path)
    for m in range(8, 16):
        ksb = qkvsb.tile([128, NL], BF16, tag="k_loc")
        proj_dT(m, ksb)
        nc.gpsimd.dma_start(out=cc_k[(m - 8) * 128:(m - 7) * 128, :], in_=ksb)
    nc.gpsimd.collective_compute(
        "AllGather", mybir.AluOpType.bypass, replica_groups=RG,
        ins=[cc_k], outs=[k_g])
    # gathered K loads on scalar (between the projection copies and the exps)
    for p in range(8):
        nc.scalar.dma_start(out=kT[p][:, 0:NL], in_=k_g[0, p * 128:(p + 1) * 128, :])
        nc.scalar.dma_start(out=kT[p][:, NL:N], in_=k_g[1, p * 128:(p + 1) * 128, :])

    pe_transpose(ld_q, wT, 0)

    # w_proj keeps the staged xbar-transpose path: it is only needed at the
    # output projection, and by now HBM and the xbar are quiet.
    ld_w = load_tiles(wproj, range(8))
    for t, i in zip(ld_w, range(8)):
        tb = prep.tile([128, D], BF16, tag="cast_bf", bufs=2, name=f"cb{i}")
        nc.vector.tensor_copy(tb, t)
        dst = bass.AP(tensor=wproj_blk.tensor,
                      offset=wproj_blk.offset + i * 128 * 128,
                      ap=[[128, 128], [wproj_blk.ap[0][0], 8], [1, 128]])
        nc.scalar.dma_start(out=dst, in_=tb.rearrange("p (k c) -> p k c", k=8))
    for k in range(8):
        nc.sync.dma_start_transpose(out=wpT[k], in_=wproj_blk[k])
    prep.release()

    # ---- Q projection (attention starts as qT tiles stream out)
    for m in range(8):
        proj_dT(m, qT[m])

    qkvsb.release()
    qkvp.release()

    # ---- Phase D: attention ----------------------------------------------
    persist2 = tc.alloc_tile_pool(name="persist2", bufs=1)
    attoutT = [persist2.tile([128, NL], BF16, tag=f"ao{p}", name=f"ao{p}") for p in range(8)]

    with tc.tile_pool(name="pT", bufs=4) as ppool, \
         tc.tile_pool(name="rcp", bufs=3) as rcpool, \
         tc.tile_pool(name="rcb", bufs=3) as rbpool, \
         tc.tile_pool(name="tmp", bufs=3) as tmppool:
        for p in range(8):
            for qc in range(2):
                qsl = slice(qc * 512, (qc + 1) * 512)
                oh = [attps.tile([65, 512], F32, tag="o_ps", bufs=4,
                                 name=f"oh{h}") for h in range(2)]
                for kt in range(16):
                    s = attps.tile([128, 2, 512], F32, tag="s_ps", bufs=2)
                    for h in range(2):
                        nc.tensor.matmul(
                            out=s[:, h, :],
                            lhsT=kT[p][h * 64:(h + 1) * 64, kt * 128:(kt + 1) * 128],
                            rhs=qT[p][h * 64:(h + 1) * 64, qsl],
                            start=True, stop=True,
                            tile_position=(h * 64, 0))
                    pt = ppool.tile([128, 2, 512], BF16, tag="pT")
                    nc.scalar.activation(pt, s, EXP, scale=SCALE)
                    vhalf = vvA if p < 4 else vvB
                    pp = p % 4
                    for h in range(2):
                        nc.tensor.matmul(
                            out=oh[h],
                            lhsT=vhalf[kt][:, pp * 130 + h * VP:pp * 130 + (h + 1) * VP],
                            rhs=pt[:, h, :],
                            start=(kt == 0), stop=(kt == 15))
                # Normalization: reciprocal of the dn row (PSUM partition 64),
                # broadcast across 64 partitions via stride-0 DMA, fused mul.
                rc = [rcpool.tile([65, 512], F32, tag=f"rc{h}", name=f"rc{h}")
                      for h in range(2)]
                rb = [rbpool.tile([64, 512], F32, tag=f"rb{h}", name=f"rb{h}")
                      for h in range(2)]
                for h in range(2):
                    nc.vector.reciprocal(rc[h][64:65, :], oh[h][64:65, :])
                    nc.gpsimd.dma_start(out=rc_d[p, qc, h], in_=rc[h][64:65, :])
                    nc.gpsimd.dma_start(out=rb[h], in_=_dram_bcast(rc_d[p, qc, h], 64))
                nc.vector.tensor_mul(attoutT[p][0:64, qsl], oh[0][0:64, :], rb[0])
                tmp = tmppool.tile([64, 512], BF16, tag="tmp")
                nc.vector.tensor_mul(tmp, oh[1][0:64, :], rb[1])
                nc.gpsimd.dma_start(out=attoutT[p][64:128, qsl], in_=tmp)

    # ---- Phase E: output projection + bias --------------------------------
    with tc.tile_pool(name="y_sb", bufs=3) as ypool:
        for tt in range(8):
            ps = attps.tile([128, 2, 512], F32, tag="s_ps", bufs=2)
            for p in range(8):
                for ec in range(2):
                    nc.tensor.matmul(
                        out=ps[:, ec, :],
                        lhsT=attoutT[p][:, tt * 128:(tt + 1) * 128],
                        rhs=wpT[p][:, ec * 512:(ec + 1) * 512],
                        start=(p == 0), stop=(p == 7))
            yt = ypool.tile([128, D], F32, tag="y_sb")
            for ec in range(2):
                nc.vector.tensor_add(yt[:, ec * 512:(ec + 1) * 512], ps[:, ec, :],
                                     bias_sb[:, ec * 512:(ec + 1) * 512])
            nc.sync.dma_start(out=out[tt * 128:(tt + 1) * 128, :], in_=yt)
    persist2.release()
    attps.release()
    persist1.release()


def _build():
    nc = bacc.Bacc("TRN2", target_bir_lowering=False, debug=False,
                   num_devices=NCORES)
    aps = {
        "x_local": nc.dram_tensor("x_local", [NL, D], F32, kind="ExternalInput").ap(),
        "w_qkv": nc.dram_tensor("w_qkv", [3 * D, D], F32, kind="ExternalInput").ap(),
        "w_proj": nc.dram_tensor("w_proj", [D, D], F32, kind="ExternalInput").ap(),
        "b_proj": nc.dram_tensor("b_proj", [D], F32, kind="ExternalInput").ap(),
        "out": nc.dram_tensor("out", [NL, D], F32, kind="ExternalOutput").ap(),
        "wproj_blk": nc.dram_tensor("wproj_blk", [8, D, 128], BF16).ap(),
        "ident_dram": nc.inline_tensor(
            np.eye(128, dtype=np.float32), name="ident_dram").ap(),
        "cc_k": nc.dram_tensor("cc_k", [D, NL], BF16).ap(),
        "cc_v": [nc.dram_tensor(f"cc_v{i}", [NL, DV // 2], BF16).ap()
                 for i in range(2)],
        "k_g": nc.dram_tensor("k_g", [2, D, NL], BF16).ap(),
        "v_g": [nc.dram_tensor(f"v_g{i}", [2, NL, DV // 2], BF16).ap()
                for i in range(2)],
        "rc_dram": nc.dram_tensor("rc_dram", [8, 2, 2, 512], F32).ap(),
    }
    with tile.TileContext(nc) as tc:
        _emit(tc, aps)
    nc.compile()
    return nc


_NC = None


def _get_nc():
    global _NC
    if _NC is None:
        _NC = _build()
    return _NC


def run(x, w_qkv, w_proj, b_proj, **spmd_kwargs):
    nc = _get_nc()
    x = np.ascontiguousarray(np.asarray(x, dtype=np.float32))
    w_qkv = np.ascontiguousarray(np.asarray(w_qkv, dtype=np.float32))
    w_proj = np.ascontiguousarray(np.asarray(w_proj, dtype=np.float32))
    b_proj = np.ascontiguousarray(np.asarray(b_proj, dtype=np.float32))
    in_maps = []
    for c in range(NCORES):
        b, half = divmod(c, 2)
        in_maps.append({
            "x_local": np.ascontiguousarray(x[b, half * NL:(half + 1) * NL, :]),
            "w_qkv": w_qkv,
            "w_proj": w_proj,
            "b_proj": b_proj,
        })
    res = run_bass_kernel_spmd(nc, in_maps, list(range(NCORES)), **spmd_kwargs)
    y = np.empty((B, N, D), dtype=np.float32)
    for c in range(NCORES):
        b, half = divmod(c, 2)
        y[b, half * NL:(half + 1) * NL, :] = res.results[c]["out"]
    return y, res


def kernel(x, w_qkv, w_proj, b_proj):
    y, _ = run(x, w_qkv, w_proj, b_proj)
    return y


# revision 31
# speedup vs baseline: 1.3374x; 1.0740x over previous
"""Multi-head attention (B=4, N=2048, D=1024, H=16) on 8 TRN2 NeuronCores.

Sharding: 8 cores = batch(4) x sequence-half(2). Each core computes the full
attention output for its 1024-token slice of one batch (all 16 heads), so the
final unshard is a pure gather. The only cross-core traffic is an AllGather of
K^T and V between the two cores of each batch pair.

Per-core pipeline (bf16 matmul operands, fp32 PSUM accumulation):
  1. Cast x / w_qkv / w_proj to bf16, stage to DRAM column-blocked, and
     DMA-transpose back so contraction dims sit on SBUF partitions.
     Loads + all collective-adjacent DMAs live on the gpsimd queue (whose
     program order matches data-readiness order); transposes on sync;
     staging stores on scalar.
  2. QKV projection. Q^T and K^T are produced in [d_out, token] orientation;
     V in natural [token, d] orientation, written into a per-head padded
     layout [V_h | 1] (65 cols per head) so the attention O-matmul also
     produces the softmax denominator.
  3. AllGather K^T then V across the pair (k-token axis spans both halves).
  4. Attention per head-pair p: S^T = (QK^T)^T via row-paired matmuls
     (contraction = head_dim 64, two heads in array row halves), exp on
     ScalarE straight out of PSUM (logits are bounded, no max subtraction),
     then one matmul per head with lhsT = [V_h | ones-col] producing
     [O_h^T; dn_h] in 65 PSUM rows - no separate denominator matmuls.
     Normalization: reciprocal of the dn row, partition-broadcast via a
     stride-0 DMA, then a single fused multiply writing bf16. The h=1 head's
     output hops through an SBUF temp + DMA to land on partitions 64-127.
  5. Output projection from the accumulated attout^T tiles, bias add, DMA out.
"""

import sys

for _p in ("/opt/trn_rl_repo",):
    if _p not in sys.path:
        sys.path.insert(0, _p)

import numpy as np

import concourse.bass as bass
import concourse.mybir as mybir
import concourse.tile as tile
from concourse import bacc
from concourse.bass_utils import run_bass_kernel_spmd

B, N, D, H, HD = 4, 2048, 1024, 16, 64
SCALE = HD ** -0.5
NL = N // 2  # tokens per core
VP = 65      # padded head width in the V layout: [V_h (64) | ones (1)]
DV = 16 * VP  # 1040
NCORES = 8
RG = [[0, 1], [2, 3], [4, 5], [6, 7]]
F32 = mybir.dt.float32
BF16 = mybir.dt.bfloat16
EXP = mybir.ActivationFunctionType.Exp


def _dram_bcast(ap_1d, n):
    """Read a contiguous DRAM row replicated across n partitions."""
    return bass.AP(tensor=ap_1d.tensor, offset=ap_1d.offset,
                   ap=[[0, n]] + list(ap_1d.ap))


def _emit(tc, aps):
    nc = tc.nc
    x_l, wqkv, wproj, bias, out = (
        aps["x_local"], aps["w_qkv"], aps["w_proj"], aps["b_proj"], aps["out"])
    wproj_blk = aps["wproj_blk"]
    cc_k, cc_v, k_g, v_g = aps["cc_k"], aps["cc_v"], aps["k_g"], aps["v_g"]
    rc_d = aps["rc_dram"]

    persist1 = tc.alloc_tile_pool(name="persist1", bufs=1)
    # Shared PSUM pool: s_ps (2 x 2 banks) serves the QKV/output projections
    # and the attention S tiles; o_ps (4 x 1 bank) holds [O_h; dn_h].
    attps = tc.alloc_tile_pool(name="att_ps", bufs=1, space="PSUM")

    # ---- Phase A: load fp32, transpose on the TensorE (transpose-mode
    # matmul against an inline identity - PE is otherwise idle here), cast
    # to bf16 in the PSUM->SBUF copy-out on VectorE. No DRAM staging
    # round-trip and no xbar-transpose serialization for x / w_qkv; only
    # w_proj (needed ~400us later) keeps the staged xbar path.
    qkvp = tc.alloc_tile_pool(name="qkvp", bufs=1)
    qkvsb = tc.alloc_tile_pool(name="qkvsb", bufs=2)
    prep = tc.alloc_tile_pool(name="prep", bufs=1)

    def load_tiles(src, tiles):
        lds = []
        for i in tiles:
            t = prep.tile([128, D], F32, tag="ld_f32", bufs=6, name=f"ld{i}")
            nc.gpsimd.dma_start(out=t, in_=src[i * 128:(i + 1) * 128, :])
            lds.append(t)
        return lds

    bias_sb = persist1.tile([128, D], F32, tag="bias")
    bias_bcast = bass.AP(tensor=bias.tensor, offset=bias.offset,
                         ap=[[0, 128], *bias.ap])
    nc.scalar.dma_start(out=bias_sb, in_=bias_bcast)

    ident = persist1.tile([128, 128], F32, tag="ident")
    nc.scalar.dma_start(out=ident, in_=aps["ident_dram"])

    def pe_transpose(lds, dst, base):
        """Transpose 8 [128, D] f32 tiles into dst[k][:, base + r*128] bf16.

        Four 128x128 transpose-mode matmuls share one PSUM bank, then one
        [128,512] VectorE copy casts to bf16. Row-quad outer so only 4 source
        tiles are live at a time.
        """
        for rq in range(2):
            for k in range(8):
                tps = attps.tile([128, 512], F32, tag="o_ps", bufs=4, name="tps")
                for j in range(4):
                    nc.tensor.transpose(
                        tps[:, j * 128:(j + 1) * 128],
                        lds[rq * 4 + j][:, k * 128:(k + 1) * 128], ident)
                nc.vector.tensor_copy(
                    dst[k][:, base + rq * 512:base + (rq + 1) * 512], tps)

    qT = [persist1.tile([128, NL], BF16, tag=f"qT{p}", name=f"qT{p}") for p in range(8)]
    kT = [persist1.tile([128, N], BF16, tag=f"kT{p}", name=f"kT{p}") for p in range(8)]
    vvA = [persist1.tile([128, DV // 2], BF16, tag=f"vA{kt}", name=f"vA{kt}")
           for kt in range(16)]
    vvB = [persist1.tile([128, DV // 2], BF16, tag=f"vB{kt}", name=f"vB{kt}")
           for kt in range(16)]
    wpT = [persist1.tile([128, D], BF16, tag=f"wpT{k}", name=f"wpT{k}")
           for k in range(8)]

    xT = [qkvp.tile([128, NL], BF16, tag=f"xT{k}", name=f"xT{k}") for k in range(8)]
    wT = [qkvp.tile([128, 3 * D], BF16, tag=f"wT{k}", name=f"wT{k}") for k in range(8)]

    # x/V/K loads issue up-front on gpsimd in consumption order; Q/wproj
    # loads are emitted later so the cc stores aren't queued behind loads
    # whose prep slots only free after the PE transposes
    ld_x = load_tiles(x_l, range(8))
    ld_v = load_tiles(wqkv, range(16, 24))   # V rows 2048:3072
    ld_k = load_tiles(wqkv, range(8, 16))    # K rows 1024:2048

    pe_transpose(ld_x, xT, 0)
    pe_transpose(ld_v, wT, 2048)

    # ---- V projection first: its AllGather (split in head-halves so each
    # 1MB piece transfers early) is the longest-latency collective.
    # Written into the padded per-head layout [V_h | 1] so the ones column
    # rides through the AllGather.
    for t in range(8):
        vsb = qkvsb.tile([128, DV], BF16, tag="v_loc")
        v4 = vsb.rearrange("p (pp h c) -> p pp h c", pp=8, h=2, c=VP)
        # full-tile memset: the projection copies overwrite everything except
        # the per-head ones column (col 64 of each 65-wide head slot)
        nc.vector.memset(vsb, 1.0)
        ps = attps.tile([128, 2, 512], F32, tag="s_ps", bufs=2)
        for k in range(8):
            for vc in range(2):
                nc.tensor.matmul(
                    out=ps[:, vc, :],
                    lhsT=xT[k][:, t * 128:(t + 1) * 128],
                    rhs=wT[k][:, 2 * D + vc * 512:2 * D + (vc + 1) * 512],
                    start=(k == 0), stop=(k == 7))
        for vc in range(2):
            nc.vector.tensor_copy(
                v4[:, vc * 4:(vc + 1) * 4, :, 0:64],
                ps[:, vc, :].rearrange("p (a h c) -> p a h c", a=4, h=2, c=64))
            cc_half = (cc_v[0] if vc == 0 else cc_v[1])
            nc.gpsimd.dma_start(
                out=cc_half[t * 128:(t + 1) * 128, :],
                in_=vsb[:, vc * (DV // 2):(vc + 1) * (DV // 2)])
    for half in range(2):
        nc.gpsimd.collective_compute(
            "AllGather", mybir.AluOpType.bypass, replica_groups=RG,
            ins=[cc_v[half]], outs=[v_g[half]])

    ld_q = load_tiles(wqkv, range(0, 8))     # Q rows 0:1024

    # gathered V loads on sync (free of transposes now)
    for kt in range(16):
        nc.sync.dma_start(
            out=vvA[kt], in_=v_g[0][kt // 8, (kt % 8) * 128:(kt % 8 + 1) * 128, :])
    for kt in range(16):
        nc.sync.dma_start(
            out=vvB[kt], in_=v_g[1][kt // 8, (kt % 8) * 128:(kt % 8 + 1) * 128, :])

    pe_transpose(ld_k, wT, 1024)

    def proj_dT(m, dst_sb):
        ps = attps.tile([128, 2, 512], F32, tag="s_ps", bufs=2)
        for k in range(8):
            for qc in range(2):
                nc.tensor.matmul(
                    out=ps[:, qc, :],
                    lhsT=wT[k][:, m * 128:(m + 1) * 128],
                    rhs=xT[k][:, qc * 512:(qc + 1) * 512],
                    start=(k == 0), stop=(k == 7))
        # single contiguous [128,1024] PSUM->SBUF copy on ScalarE (Copy is in
        # every ACT table set, so no table reload before the exps)
        nc.scalar.copy(dst_sb, ps.rearrange("p a c -> p (a c)"))

    # ---- K projection + AllGather (fast mesh path)
    for m in range(8, 16):
        ksb = qkvsb.tile([128, NL], BF16, tag="k_loc", bufs=3)
        proj_dT(m, ksb)
        nc.gpsimd.dma_start(out=cc_k[(m - 8) * 128:(m - 7) * 128, :], in_=ksb)
    nc.gpsimd.collective_compute(
        "AllGather", mybir.AluOpType.bypass, replica_groups=RG,
        ins=[cc_k], outs=[k_g])
    # gathered K loads on sync (idle once the V gathers drain)
    for p in range(8):
        nc.sync.dma_start(out=kT[p][:, 0:NL], in_=k_g[0, p * 128:(p + 1) * 128, :])
        nc.sync.dma_start(out=kT[p][:, NL:N], in_=k_g[1, p * 128:(p + 1) * 128, :])

    pe_transpose(ld_q, wT, 0)

    # w_proj keeps the staged xbar-transpose path: it is only needed at the
    # output projection, and by now HBM and the xbar are quiet.
    ld_w = load_tiles(wproj, range(8))
    for t, i in zip(ld_w, range(8)):
        tb = prep.tile([128, D], BF16, tag="cast_bf", bufs=2, name=f"cb{i}")
        nc.vector.tensor_copy(tb, t)
        dst = bass.AP(tensor=wproj_blk.tensor,
                      offset=wproj_blk.offset + i * 128 * 128,
                      ap=[[128, 128], [wproj_blk.ap[0][0], 8], [1, 128]])
        nc.scalar.dma_start(out=dst, in_=tb.rearrange("p (k c) -> p k c", k=8))
    for k in range(8):
        nc.sync.dma_start_transpose(out=wpT[k], in_=wproj_blk[k])
    prep.release()

    # ---- Q projection (attention starts as qT tiles stream out)
    for m in range(8):
        proj_dT(m, qT[m])

    qkvsb.release()
    qkvp.release()

    # ---- Phase D: attention ----------------------------------------------
    persist2 = tc.alloc_tile_pool(name="persist2", bufs=1)
    attoutT = [persist2.tile([128, NL], BF16, tag=f"ao{p}", name=f"ao{p}") for p in range(8)]

    with tc.tile_pool(name="pT", bufs=4) as ppool, \
         tc.tile_pool(name="rcp", bufs=3) as rcpool, \
         tc.tile_pool(name="rcb", bufs=3) as rbpool, \
         tc.tile_pool(name="tmp", bufs=3) as tmppool:
        for p in range(8):
            for qc in range(2):
                qsl = slice(qc * 512, (qc + 1) * 512)
                oh = [attps.tile([65, 512], F32, tag="o_ps", bufs=4,
                                 name=f"oh{h}") for h in range(2)]
                for kt in range(16):
                    s = attps.tile([128, 2, 512], F32, tag="s_ps", bufs=2)
                    for h in range(2):
                        nc.tensor.matmul(
                            out=s[:, h, :],
                            lhsT=kT[p][h * 64:(h + 1) * 64, kt * 128:(kt + 1) * 128],
                            rhs=qT[p][h * 64:(h + 1) * 64, qsl],
                            start=True, stop=True,
                            tile_position=(h * 64, 0))
                    pt = ppool.tile([128, 2, 512], BF16, tag="pT")
                    nc.scalar.activation(pt, s, EXP, scale=SCALE)
                    vhalf = vvA if p < 4 else vvB
                    pp = p % 4
                    for h in range(2):
                        nc.tensor.matmul(
                            out=oh[h],
                            lhsT=vhalf[kt][:, pp * 130 + h * VP:pp * 130 + (h + 1) * VP],
                            rhs=pt[:, h, :],
                            start=(kt == 0), stop=(kt == 15))
                # Normalization: reciprocal of the dn row (PSUM partition 64),
                # broadcast across 64 partitions via stride-0 DMA, fused mul.
                rc = [rcpool.tile([65, 512], F32, tag=f"rc{h}", name=f"rc{h}")
                      for h in range(2)]
                rb = [rbpool.tile([64, 512], F32, tag=f"rb{h}", name=f"rb{h}")
                      for h in range(2)]
                for h in range(2):
                    nc.vector.reciprocal(rc[h][64:65, :], oh[h][64:65, :])
                    nc.gpsimd.dma_start(out=rc_d[p, qc, h], in_=rc[h][64:65, :])
                    nc.gpsimd.dma_start(out=rb[h], in_=_dram_bcast(rc_d[p, qc, h], 64))
                nc.vector.tensor_mul(attoutT[p][0:64, qsl], oh[0][0:64, :], rb[0])
                tmp = tmppool.tile([64, 512], BF16, tag="tmp")
                nc.vector.tensor_mul(tmp, oh[1][0:64, :], rb[1])
                nc.gpsimd.dma_start(out=attoutT[p][64:128, qsl], in_=tmp)

    # ---- Phase E: output projection + bias --------------------------------
    with tc.tile_pool(name="y_sb", bufs=3) as ypool:
        for tt in range(8):
            ps = attps.tile([128, 2, 512], F32, tag="s_ps", bufs=2)
            for p in range(8):
                for ec in range(2):
                    nc.tensor.matmul(
                        out=ps[:, ec, :],
                        lhsT=attoutT[p][:, tt * 128:(tt + 1) * 128],
                        rhs=wpT[p][:, ec * 512:(ec + 1) * 512],
                        start=(p == 0), stop=(p == 7))
            yt = ypool.tile([128, D], F32, tag="y_sb")
            for ec in range(2):
                nc.vector.tensor_add(yt[:, ec * 512:(ec + 1) * 512], ps[:, ec, :],
                                     bias_sb[:, ec * 512:(ec + 1) * 512])
            nc.sync.dma_start(out=out[tt * 128:(tt + 1) * 128, :], in_=yt)
    persist2.release()
    attps.release()
    persist1.release()


def _build():
    nc = bacc.Bacc("TRN2", target_bir_lowering=False, debug=False,
                   num_devices=NCORES)
    aps = {
        "x_local": nc.dram_tensor("x_local", [NL, D], F32, kind="ExternalInput").ap(),
        "w_qkv": nc.dram_tensor("w_qkv", [3 * D, D], F32, kind="ExternalInput").ap(),
        "w_proj": nc.dram_tensor("w_proj", [D, D], F32, kind="ExternalInput").ap(),
        "b_proj": nc.dram_tensor("b_proj", [D], F32, kind="ExternalInput").ap(),
        "out": nc.dram_tensor("out", [NL, D], F32, kind="ExternalOutput").ap(),
        "wproj_blk": nc.dram_tensor("wproj_blk", [8, D, 128], BF16).ap(),
        "ident_dram": nc.inline_tensor(
            np.eye(128, dtype=np.float32), name="ident_dram").ap(),
        "cc_k": nc.dram_tensor("cc_k", [D, NL], BF16).ap(),
        "cc_v": [nc.dram_tensor(f"cc_v{i}", [NL, DV // 2], BF16).ap()
                 for i in range(2)],
        "k_g": nc.dram_tensor("k_g", [2, D, NL], BF16).ap(),
        "v_g": [nc.dram_tensor(f"v_g{i}", [2, NL, DV // 2], BF16).ap()
                for i in range(2)],
        "rc_dram": nc.dram_tensor("rc_dram", [8, 2, 2, 512], F32).ap(),
    }
    with tile.TileContext(nc) as tc:
        _emit(tc, aps)
    nc.compile()
    return nc


_NC = None


def _get_nc():
    global _NC
    if _NC is None:
        _NC = _build()
    return _NC


def run(x, w_qkv, w_proj, b_proj, **spmd_kwargs):
    nc = _get_nc()
    x = np.ascontiguousarray(np.asarray(x, dtype=np.float32))
    w_qkv = np.ascontiguousarray(np.asarray(w_qkv, dtype=np.float32))
    w_proj = np.ascontiguousarray(np.asarray(w_proj, dtype=np.float32))
    b_proj = np.ascontiguousarray(np.asarray(b_proj, dtype=np.float32))
    in_maps = []
    for c in range(NCORES):
        b, half = divmod(c, 2)
        in_maps.append({
            "x_local": np.ascontiguousarray(x[b, half * NL:(half + 1) * NL, :]),
            "w_qkv": w_qkv,
            "w_proj": w_proj,
            "b_proj": b_proj,
        })
    res = run_bass_kernel_spmd(nc, in_maps, list(range(NCORES)), **spmd_kwargs)
    y = np.empty((B, N, D), dtype=np.float32)
    for c in range(NCORES):
        b, half = divmod(c, 2)
        y[b, half * NL:(half + 1) * NL, :] = res.results[c]["out"]
    return y, res


def kernel(x, w_qkv, w_proj, b_proj):
    y, _ = run(x, w_qkv, w_proj, b_proj)
    return y


# revision 35
# speedup vs baseline: 1.3405x; 1.0024x over previous
"""Multi-head attention (B=4, N=2048, D=1024, H=16) on 8 TRN2 NeuronCores.

Sharding: 8 cores = batch(4) x sequence-half(2). Each core computes the full
attention output for its 1024-token slice of one batch (all 16 heads), so the
final unshard is a pure gather. The only cross-core traffic is an AllGather of
K^T and V between the two cores of each batch pair.

Per-core pipeline (bf16 matmul operands, fp32 PSUM accumulation):
  1. Cast x / w_qkv / w_proj to bf16, stage to DRAM column-blocked, and
     DMA-transpose back so contraction dims sit on SBUF partitions.
     Loads + all collective-adjacent DMAs live on the gpsimd queue (whose
     program order matches data-readiness order); transposes on sync;
     staging stores on scalar.
  2. QKV projection. Q^T and K^T are produced in [d_out, token] orientation;
     V in natural [token, d] orientation, written into a per-head padded
     layout [V_h | 1] (65 cols per head) so the attention O-matmul also
     produces the softmax denominator.
  3. AllGather K^T then V across the pair (k-token axis spans both halves).
  4. Attention per head-pair p: S^T = (QK^T)^T via row-paired matmuls
     (contraction = head_dim 64, two heads in array row halves), exp on
     ScalarE straight out of PSUM (logits are bounded, no max subtraction),
     then one matmul per head with lhsT = [V_h | ones-col] producing
     [O_h^T; dn_h] in 65 PSUM rows - no separate denominator matmuls.
     Normalization: reciprocal of the dn row, partition-broadcast via a
     stride-0 DMA, then a single fused multiply writing bf16. The h=1 head's
     output hops through an SBUF temp + DMA to land on partitions 64-127.
  5. Output projection from the accumulated attout^T tiles, bias add, DMA out.
"""

import sys

for _p in ("/opt/trn_rl_repo",):
    if _p not in sys.path:
        sys.path.insert(0, _p)

import numpy as np

import concourse.bass as bass
import concourse.mybir as mybir
import concourse.tile as tile
from concourse import bacc
from concourse.bass_utils import run_bass_kernel_spmd

B, N, D, H, HD = 4, 2048, 1024, 16, 64
SCALE = HD ** -0.5
NL = N // 2  # tokens per core
VP = 65      # padded head width in the V layout: [V_h (64) | ones (1)]
DV = 16 * VP  # 1040
NCORES = 8
RG = [[0, 1], [2, 3], [4, 5], [6, 7]]
F32 = mybir.dt.float32
BF16 = mybir.dt.bfloat16
EXP = mybir.ActivationFunctionType.Exp


def _dram_bcast(ap_1d, n):
    """Read a contiguous DRAM row replicated across n partitions."""
    return bass.AP(tensor=ap_1d.tensor, offset=ap_1d.offset,
                   ap=[[0, n]] + list(ap_1d.ap))


def _emit(tc, aps):
    nc = tc.nc
    x_l, wqkv, wproj, bias, out = (
        aps["x_local"], aps["w_qkv"], aps["w_proj"], aps["b_proj"], aps["out"])
    wproj_blk = aps["wproj_blk"]
    cc_k, cc_v, k_g, v_g = aps["cc_k"], aps["cc_v"], aps["k_g"], aps["v_g"]
    rc_d = aps["rc_dram"]

    persist1 = tc.alloc_tile_pool(name="persist1", bufs=1)
    # Shared PSUM pool: s_ps (2 x 2 banks) serves the QKV/output projections
    # and the attention S tiles; o_ps (4 x 1 bank) holds [O_h; dn_h].
    attps = tc.alloc_tile_pool(name="att_ps", bufs=1, space="PSUM")

    # ---- Phase A: load fp32, transpose on the TensorE (transpose-mode
    # matmul against an inline identity - PE is otherwise idle here), cast
    # to bf16 in the PSUM->SBUF copy-out on VectorE. No DRAM staging
    # round-trip and no xbar-transpose serialization for x / w_qkv; only
    # w_proj (needed ~400us later) keeps the staged xbar path.
    qkvp = tc.alloc_tile_pool(name="qkvp", bufs=1)
    qkvsb = tc.alloc_tile_pool(name="qkvsb", bufs=2)
    prep = tc.alloc_tile_pool(name="prep", bufs=1)

    def load_tiles(src, tiles):
        lds = []
        for i in tiles:
            t = prep.tile([128, D], F32, tag="ld_f32", bufs=6, name=f"ld{i}")
            nc.gpsimd.dma_start(out=t, in_=src[i * 128:(i + 1) * 128, :])
            lds.append(t)
        return lds

    bias_sb = persist1.tile([128, D], F32, tag="bias")
    bias_bcast = bass.AP(tensor=bias.tensor, offset=bias.offset,
                         ap=[[0, 128], *bias.ap])
    nc.scalar.dma_start(out=bias_sb, in_=bias_bcast)

    ident = persist1.tile([128, 128], F32, tag="ident")
    nc.scalar.dma_start(out=ident, in_=aps["ident_dram"])

    def pe_transpose(lds, dst, base):
        """Transpose 8 [128, D] f32 tiles into dst[k][:, base + r*128] bf16.

        Four 128x128 transpose-mode matmuls share one PSUM bank, then one
        [128,512] VectorE copy casts to bf16. Row-quad outer so only 4 source
        tiles are live at a time.
        """
        for rq in range(2):
            for k in range(8):
                tps = attps.tile([128, 512], F32, tag="o_ps", bufs=4, name="tps")
                for j in range(4):
                    nc.tensor.transpose(
                        tps[:, j * 128:(j + 1) * 128],
                        lds[rq * 4 + j][:, k * 128:(k + 1) * 128], ident)
                nc.vector.tensor_copy(
                    dst[k][:, base + rq * 512:base + (rq + 1) * 512], tps)

    qT = [persist1.tile([128, NL], BF16, tag=f"qT{p}", name=f"qT{p}") for p in range(8)]
    kT = [persist1.tile([128, N], BF16, tag=f"kT{p}", name=f"kT{p}") for p in range(8)]
    vvA = [persist1.tile([128, DV // 2], BF16, tag=f"vA{kt}", name=f"vA{kt}")
           for kt in range(16)]
    vvB = [persist1.tile([128, DV // 2], BF16, tag=f"vB{kt}", name=f"vB{kt}")
           for kt in range(16)]
    wpT = [persist1.tile([128, D], BF16, tag=f"wpT{k}", name=f"wpT{k}")
           for k in range(8)]

    xT = [qkvp.tile([128, NL], BF16, tag=f"xT{k}", name=f"xT{k}") for k in range(8)]
    wT = [qkvp.tile([128, 3 * D], BF16, tag=f"wT{k}", name=f"wT{k}") for k in range(8)]

    # x/K/V loads issue up-front on gpsimd in consumption order; Q/wproj
    # loads are emitted later so the cc stores aren't queued behind loads
    # whose prep slots only free after the PE transposes
    ld_x = load_tiles(x_l, range(8))
    ld_k = load_tiles(wqkv, range(8, 16))    # K rows 1024:2048
    ld_v = load_tiles(wqkv, range(16, 24))   # V rows 2048:3072

    pe_transpose(ld_x, xT, 0)
    pe_transpose(ld_k, wT, 1024)

    def proj_dT(m, dst_sb):
        ps = attps.tile([128, 2, 512], F32, tag="s_ps", bufs=2)
        for k in range(8):
            for qc in range(2):
                nc.tensor.matmul(
                    out=ps[:, qc, :],
                    lhsT=wT[k][:, m * 128:(m + 1) * 128],
                    rhs=xT[k][:, qc * 512:(qc + 1) * 512],
                    start=(k == 0), stop=(k == 7))
        # single contiguous [128,1024] PSUM->SBUF copy on ScalarE (Copy is in
        # every ACT table set, so no table reload before the exps)
        nc.scalar.copy(dst_sb, ps.rearrange("p a c -> p (a c)"))

    # ---- K projection first + AllGather: kT gates the attention S matmuls,
    # so its collective must launch as early as possible.
    for m in range(8, 16):
        ksb = qkvsb.tile([128, NL], BF16, tag="k_loc", bufs=3)
        proj_dT(m, ksb)
        nc.gpsimd.dma_start(out=cc_k[(m - 8) * 128:(m - 7) * 128, :], in_=ksb)
    nc.gpsimd.collective_compute(
        "AllGather", mybir.AluOpType.bypass, replica_groups=RG,
        ins=[cc_k], outs=[k_g])
    # gathered K loads on sync
    for p in range(8):
        nc.sync.dma_start(out=kT[p][:, 0:NL], in_=k_g[0, p * 128:(p + 1) * 128, :])
        nc.sync.dma_start(out=kT[p][:, NL:N], in_=k_g[1, p * 128:(p + 1) * 128, :])

    ld_q = load_tiles(wqkv, range(0, 8))     # Q rows 0:1024

    pe_transpose(ld_v, wT, 2048)

    # ---- V projection + head-split AllGather (vv tiles are only needed
    # once the first exp completes, ~15us after the S matmuls start).
    # Written into the padded per-head layout [V_h | 1] so the ones column
    # rides through the AllGather.
    for t in range(8):
        vsb = qkvsb.tile([128, DV], BF16, tag="v_loc")
        v4 = vsb.rearrange("p (pp h c) -> p pp h c", pp=8, h=2, c=VP)
        # full-tile memset: the projection copies overwrite everything except
        # the per-head ones column (col 64 of each 65-wide head slot)
        nc.vector.memset(vsb, 1.0)
        ps = attps.tile([128, 2, 512], F32, tag="s_ps", bufs=2)
        for k in range(8):
            for vc in range(2):
                nc.tensor.matmul(
                    out=ps[:, vc, :],
                    lhsT=xT[k][:, t * 128:(t + 1) * 128],
                    rhs=wT[k][:, 2 * D + vc * 512:2 * D + (vc + 1) * 512],
                    start=(k == 0), stop=(k == 7))
        for vc in range(2):
            nc.vector.tensor_copy(
                v4[:, vc * 4:(vc + 1) * 4, :, 0:64],
                ps[:, vc, :].rearrange("p (a h c) -> p a h c", a=4, h=2, c=64))
            cc_half = (cc_v[0] if vc == 0 else cc_v[1])
            nc.gpsimd.dma_start(
                out=cc_half[t * 128:(t + 1) * 128, :],
                in_=vsb[:, vc * (DV // 2):(vc + 1) * (DV // 2)])
    for half in range(2):
        nc.gpsimd.collective_compute(
            "AllGather", mybir.AluOpType.bypass, replica_groups=RG,
            ins=[cc_v[half]], outs=[v_g[half]])

    # gathered V loads on sync (after the kT gathers)
    for kt in range(16):
        nc.sync.dma_start(
            out=vvA[kt], in_=v_g[0][kt // 8, (kt % 8) * 128:(kt % 8 + 1) * 128, :])
    for kt in range(16):
        nc.sync.dma_start(
            out=vvB[kt], in_=v_g[1][kt // 8, (kt % 8) * 128:(kt % 8 + 1) * 128, :])

    pe_transpose(ld_q, wT, 0)

    # w_proj keeps the staged xbar-transpose path: it is only needed at the
    # output projection, and by now HBM and the xbar are quiet.
    ld_w = load_tiles(wproj, range(8))
    for t, i in zip(ld_w, range(8)):
        tb = prep.tile([128, D], BF16, tag="cast_bf", bufs=2, name=f"cb{i}")
        nc.vector.tensor_copy(tb, t)
        dst = bass.AP(tensor=wproj_blk.tensor,
                      offset=wproj_blk.offset + i * 128 * 128,
                      ap=[[128, 128], [wproj_blk.ap[0][0], 8], [1, 128]])
        nc.scalar.dma_start(out=dst, in_=tb.rearrange("p (k c) -> p k c", k=8))
    for k in range(8):
        nc.sync.dma_start_transpose(out=wpT[k], in_=wproj_blk[k])
    prep.release()

    # ---- Q projection (attention starts as qT tiles stream out)
    for m in range(8):
        proj_dT(m, qT[m])

    qkvsb.release()
    qkvp.release()

    # ---- Phase D: attention ----------------------------------------------
    persist2 = tc.alloc_tile_pool(name="persist2", bufs=1)
    attoutT = [persist2.tile([128, NL], BF16, tag=f"ao{p}", name=f"ao{p}") for p in range(8)]

    with tc.tile_pool(name="pT", bufs=4) as ppool, \
         tc.tile_pool(name="rcp", bufs=3) as rcpool, \
         tc.tile_pool(name="rcb", bufs=3) as rbpool, \
         tc.tile_pool(name="tmp", bufs=3) as tmppool:
        for p in range(8):
            for qc in range(2):
                qsl = slice(qc * 512, (qc + 1) * 512)
                oh = [attps.tile([65, 512], F32, tag="o_ps", bufs=4,
                                 name=f"oh{h}") for h in range(2)]
                for kt in range(16):
                    s = attps.tile([128, 2, 512], F32, tag="s_ps", bufs=2)
                    for h in range(2):
                        nc.tensor.matmul(
                            out=s[:, h, :],
                            lhsT=kT[p][h * 64:(h + 1) * 64, kt * 128:(kt + 1) * 128],
                            rhs=qT[p][h * 64:(h + 1) * 64, qsl],
                            start=True, stop=True,
                            tile_position=(h * 64, 0))
                    pt = ppool.tile([128, 2, 512], BF16, tag="pT")
                    nc.scalar.activation(pt, s, EXP, scale=SCALE)
                    vhalf = vvA if p < 4 else vvB
                    pp = p % 4
                    for h in range(2):
                        nc.tensor.matmul(
                            out=oh[h],
                            lhsT=vhalf[kt][:, pp * 130 + h * VP:pp * 130 + (h + 1) * VP],
                            rhs=pt[:, h, :],
                            start=(kt == 0), stop=(kt == 15))
                # Normalization: reciprocal of the dn row (PSUM partition 64),
                # broadcast across 64 partitions via stride-0 DMA, fused mul.
                rc = [rcpool.tile([65, 512], F32, tag=f"rc{h}", name=f"rc{h}")
                      for h in range(2)]
                rb = [rbpool.tile([64, 512], F32, tag=f"rb{h}", name=f"rb{h}")
                      for h in range(2)]
                for h in range(2):
                    nc.vector.reciprocal(rc[h][64:65, :], oh[h][64:65, :])
                    nc.gpsimd.dma_start(out=rc_d[p, qc, h], in_=rc[h][64:65, :])
                    nc.gpsimd.dma_start(out=rb[h], in_=_dram_bcast(rc_d[p, qc, h], 64))
                nc.vector.tensor_mul(attoutT[p][0:64, qsl], oh[0][0:64, :], rb[0])
                tmp = tmppool.tile([64, 512], BF16, tag="tmp")
                nc.vector.tensor_mul(tmp, oh[1][0:64, :], rb[1])
                nc.gpsimd.dma_start(out=attoutT[p][64:128, qsl], in_=tmp)

    # ---- Phase E: output projection + bias --------------------------------
    with tc.tile_pool(name="y_sb", bufs=3) as ypool:
        for tt in range(8):
            ps = attps.tile([128, 2, 512], F32, tag="s_ps", bufs=2)
            for p in range(8):
                for ec in range(2):
                    nc.tensor.matmul(
                        out=ps[:, ec, :],
                        lhsT=attoutT[p][:, tt * 128:(tt + 1) * 128],
                        rhs=wpT[p][:, ec * 512:(ec + 1) * 512],
                        start=(p == 0), stop=(p == 7))
            yt = ypool.tile([128, D], F32, tag="y_sb")
            for ec in range(2):
                nc.vector.tensor_add(yt[:, ec * 512:(ec + 1) * 512], ps[:, ec, :],
                                     bias_sb[:, ec * 512:(ec + 1) * 512])
            nc.sync.dma_start(out=out[tt * 128:(tt + 1) * 128, :], in_=yt)
    persist2.release()
    attps.release()
    persist1.release()


def _build():
    nc = bacc.Bacc("TRN2", target_bir_lowering=False, debug=False,
                   num_devices=NCORES)
    aps = {
        "x_local": nc.dram_tensor("x_local", [NL, D], F32, kind="ExternalInput").ap(),
        "w_qkv": nc.dram_tensor("w_qkv", [3 * D, D], F32, kind="ExternalInput").ap(),
        "w_proj": nc.dram_tensor("w_proj", [D, D], F32, kind="ExternalInput").ap(),
        "b_proj": nc.dram_tensor("b_proj", [D], F32, kind="ExternalInput").ap(),
        "out": nc.dram_tensor("out", [NL, D], F32, kind="ExternalOutput").ap(),
        "wproj_blk": nc.dram_tensor("wproj_blk", [8, D, 128], BF16).ap(),
        "ident_dram": nc.inline_tensor(
            np.eye(128, dtype=np.float32), name="ident_dram").ap(),
        "cc_k": nc.dram_tensor("cc_k", [D, NL], BF16).ap(),
        "cc_v": [nc.dram_tensor(f"cc_v{i}", [NL, DV // 2], BF16).ap()
                 for i in range(2)],
        "k_g": nc.dram_tensor("k_g", [2, D, NL], BF16).ap(),
        "v_g": [nc.dram_tensor(f"v_g{i}", [2, NL, DV // 2], BF16).ap()
                for i in range(2)],
        "rc_dram": nc.dram_tensor("rc_dram", [8, 2, 2, 512], F32).ap(),
    }
    with tile.TileContext(nc) as tc:
        _emit(tc, aps)
    nc.compile()
    return nc


_NC = None


def _get_nc():
    global _NC
    if _NC is None:
        _NC = _build()
    return _NC


def run(x, w_qkv, w_proj, b_proj, **spmd_kwargs):
    nc = _get_nc()
    x = np.ascontiguousarray(np.asarray(x, dtype=np.float32))
    w_qkv = np.ascontiguousarray(np.asarray(w_qkv, dtype=np.float32))
    w_proj = np.ascontiguousarray(np.asarray(w_proj, dtype=np.float32))
    b_proj = np.ascontiguousarray(np.asarray(b_proj, dtype=np.float32))
    in_maps = []
    for c in range(NCORES):
        b, half = divmod(c, 2)
        in_maps.append({
            "x_local": np.ascontiguousarray(x[b, half * NL:(half + 1) * NL, :]),
            "w_qkv": w_qkv,
            "w_proj": w_proj,
            "b_proj": b_proj,
        })
    res = run_bass_kernel_spmd(nc, in_maps, list(range(NCORES)), **spmd_kwargs)
    y = np.empty((B, N, D), dtype=np.float32)
    for c in range(NCORES):
        b, half = divmod(c, 2)
        y[b, half * NL:(half + 1) * NL, :] = res.results[c]["out"]
    return y, res


def kernel(x, w_qkv, w_proj, b_proj):
    y, _ = run(x, w_qkv, w_proj, b_proj)
    return y
